# revision 3
# baseline (speedup 1.0000x reference)
"""Distributed causal multi-head attention for 8 TRN2 NeuronCores.

Problem: y = (softmax(mask(Q K^T / sqrt(d))) V) @ c_proj_w + c_proj_b with
Q,K,V = split(x @ c_attn_w + c_attn_b), shapes B=2, S=2048, NX=1024, NH=16,
HD=64.

Sharding: core c = (b, g) with b = c // 4, g = c % 4 — data parallel over the
batch, tensor parallel over 4 head-groups of 4 heads. Each core:
  1. computes qT/kT ([d, s] layout) and v ([s, d] layout) for its 4 heads from
     a host-pretransposed x[b]^T, so no on-device transposes are ever needed;
  2. runs causal attention in the "S^T" orientation: scores come out of the PE
     as [j, i] tiles, exp() is fused into the PSUM->SBUF copy on the scalar
     engine (no max-subtraction — scores are bounded), and the softmax
     denominator falls out of the PV matmul for free via a ones-column
     appended to V;
  3. AllGathers aT per head-pair piece as soon as the pair finishes, and
     computes a 256-wide slice of the output projection in the transposed
     orientation (out^T = wp^T aT, [f, s] tiles), so each AG piece is exactly
     one 128-row contraction block and proj streams full 512-wide columns.
The host wrapper only slices/transposes inputs and concatenates outputs.
"""

import ml_dtypes
import numpy as np

import concourse.bass as bass
import concourse.mybir as mybir
from concourse import bacc, tile
from concourse.bass_utils import run_bass_kernel_spmd

B, S, NX, NH, HD = 2, 2048, 1024, 16, 64
NG = 4              # head-groups == cores per batch entry
HG = NH // NG       # heads per core
FG = HG * HD        # local feature width (256)
P = 128
SC = 512            # sequence chunk width
NSC = S // SC       # 4 chunks
KO = NX // P        # 8 contraction tiles
VW = HD + 1         # v tile width: 64 data cols + ones column
N_CORES = 8

F32 = mybir.dt.float32
MM_DT = mybir.dt.bfloat16

REPLICA_GROUPS = [[0, 1, 2, 3], [4, 5, 6, 7]]


def build(nc: bass.Bass):
    xT = nc.declare_dram_parameter("xT", [NX, S], MM_DT, isOutput=False)
    wq = nc.declare_dram_parameter("wq", [NX, FG], MM_DT, isOutput=False)
    wk = nc.declare_dram_parameter("wk", [NX, FG], MM_DT, isOutput=False)
    wv = nc.declare_dram_parameter("wv", [NX, FG], MM_DT, isOutput=False)
    bqk = nc.declare_dram_parameter("bqk", [P, 4], F32, isOutput=False)
    bv = nc.declare_dram_parameter("bv", [P, FG], MM_DT, isOutput=False)
    wp = nc.declare_dram_parameter("wp", [NX, FG], MM_DT, isOutput=False)
    bp = nc.declare_dram_parameter("bp", [P, 2], F32, isOutput=False)
    maskw = nc.declare_dram_parameter("maskw", [P, P], MM_DT, isOutput=False)
    out = nc.declare_dram_parameter("out", [FG, S], F32, isOutput=True)

    # Per-piece collective bounce buffers (collectives can't touch kernel I/O).
    # Piece (sc, pc) carries heads {2pc, 2pc+1} of chunk sc; after the
    # 4-rank AllGather, rank r's rows are global heads {4r+2pc, 4r+2pc+1}
    # == contraction tile ko = 2r+pc of the output projection.
    ag_warm_in = nc.dram_tensor("ag_warm_in", [4, 128], MM_DT)
    ag_warm_out = nc.dram_tensor("ag_warm_out", [16, 128], MM_DT)
    aT_loc = [[nc.dram_tensor(f"aT_loc{c}_{p_}", [P, SC], MM_DT)
               for p_ in range(2)] for c in range(NSC)]
    aT_full = [[nc.dram_tensor(f"aT_full{c}_{p_}", [NG * P, SC], MM_DT)
                for p_ in range(2)] for c in range(NSC)]

    with tile.TileContext(nc) as tc:
        nc_lp = nc.allow_low_precision(reason="bf16 PE compute path")
        nc_lp.__enter__()
        with (
            tc.tile_pool(name="consts", bufs=1) as consts,
            tc.tile_pool(name="persist", bufs=1) as persist,
            tc.tile_pool(name="xt", bufs=4) as xt_pool,
            tc.tile_pool(name="pt", bufs=8) as pt_pool,
            tc.tile_pool(name="aTf", bufs=2) as aTf_pool,
            tc.tile_pool(name="outs", bufs=3) as out_pool,
            tc.tile_pool(name="small", bufs=2) as small,
            tc.tile_pool(name="psum", bufs=2, space="PSUM") as psum,
        ):
            wq_sb = consts.tile([P, KO, FG], MM_DT)
            wk_sb = consts.tile([P, KO, FG], MM_DT)
            wv_sb = consts.tile([P, KO, FG], MM_DT)
            wp_sb = consts.tile([P, KO, FG], MM_DT)
            bqk_sb = consts.tile([P, 4], F32)
            bv_sb = consts.tile([P, FG], MM_DT)
            bp_sb = consts.tile([P, 2], F32)
            maskw_sb = consts.tile([P, P], MM_DT)
            ones64 = consts.tile([1, HD], MM_DT)

            # ---- startup: collective warm-up first on the CC path, then
            # the chunk-0-critical DMAs spread across engine queues so the
            # first QKV matmul isn't serialized behind a single ring ----
            nc.gpsimd.collective_compute(
                "AllGather",
                mybir.AluOpType.bypass,
                ins=[ag_warm_in[:].opt()],
                outs=[ag_warm_out[:].opt()],
                replica_groups=REPLICA_GROUPS,
            )
            xts = []
            for sc in range(NSC):
                xt = xt_pool.tile([P, KO, SC], MM_DT, tag="xt", name=f"xt{sc}")
                xts.append(xt)
            nc.sync.dma_start(
                xts[0][:], xT.rearrange("(ko p) s -> p ko s", p=P)[:, :, 0:SC]
            )
            nc.scalar.dma_start(wq_sb[:], wq.rearrange("(ko p) f -> p ko f", p=P))
            nc.scalar.dma_start(wk_sb[:], wk.rearrange("(ko p) f -> p ko f", p=P))
            nc.gpsimd.dma_start(wv_sb[:], wv.rearrange("(ko p) f -> p ko f", p=P))
            nc.scalar.dma_start(bqk_sb[:], bqk[:])
            nc.gpsimd.dma_start(bv_sb[:], bv[:])
            for sc in range(1, NSC):
                nc.sync.dma_start(
                    xts[sc][:],
                    xT.rearrange("(ko p) s -> p ko s", p=P)[:, :, sc * SC:(sc + 1) * SC],
                )
            nc.gpsimd.dma_start(maskw_sb[:], maskw[:])
            nc.gpsimd.dma_start(wp_sb[:], wp.rearrange("(ko p) f -> p ko f", p=P))
            nc.gpsimd.dma_start(bp_sb[:], bp[:])
            nc.vector.memset(ones64[:], 1.0)

            # ---- persistent activation tiles ----
            # kT: [d, s] packed — tile hh holds heads (2hh, 2hh+1) on
            # partition halves; it is the scores lhsT.
            # qT: one zero-padded [128, s] tile per head, data on the same
            # partition half as in kT, zeros elsewhere — the zeros select
            # the head out of the packed kT during the scores matmul.
            # v: [s, 65] per head per 128-row tile: 64 data cols + a ones
            # column (softmax denominator); the PV lhsT is [128, 65] so no
            # zero padding is ever needed.
            # aT: per-head [128, s]; only rows 0:64 are meaningful.
            qT_sb = [persist.tile([P, S], MM_DT, name=f"qT{h}") for h in range(HG)]
            kT_sb = [persist.tile([P, S], MM_DT, name=f"kT{hh}") for hh in range(2)]
            v_sb = [persist.tile([P, HG, VW], MM_DT, name=f"v{st}")
                    for st in range(S // P)]
            aT_sb = [persist.tile([P, S], MM_DT, name=f"aT{h}") for h in range(HG)]
            for h in range(HG):
                pad0 = (1 - h % 2) * HD
                nc.vector.memset(qT_sb[h][pad0:pad0 + HD, :], 0.0)

            def proj_chunk(sc):
                aTfp = []
                for pc in range(2):
                    t = aTf_pool.tile([P, NG, SC], MM_DT, tag=f"aTfp{pc}",
                                      name=f"aTfp{pc}")
                    nc.sync.dma_start(
                        t[:], aT_full[sc][pc].rearrange("(r p) s -> p r s", p=P)
                    )
                    aTfp.append(t)
                ps_ft = [psum.tile([P, SC], F32, tag="proj", name=f"proj{ft}")
                         for ft in range(2)]
                for pc in range(2):
                    for ft in range(2):
                        for r in range(NG):
                            nc.tensor.matmul(
                                ps_ft[ft][:],
                                wp_sb[:, 2 * r + pc, ft * P:(ft + 1) * P],
                                aTfp[pc][:, r, :],
                                start=(pc == 0 and r == 0),
                                stop=(pc == 1 and r == NG - 1),
                            )
                for ft in range(2):
                    ot = out_pool.tile([P, SC], F32, tag="ot")
                    nc.vector.tensor_scalar_add(
                        ot[:], ps_ft[ft][:], bp_sb[:, ft:ft + 1]
                    )
                    nc.sync.dma_start(
                        out[ft * P:(ft + 1) * P, sc * SC:(sc + 1) * SC], ot[:]
                    )

            # ===== per-chunk pipeline: QKV -> attention -> AllGather pieces
            # -> (deferred one chunk) output projection =====
            for sc in range(NSC):
                # ---- QKV for this chunk ----
                xt = xts[sc]
                for qk, w_sb in enumerate((wq_sb, wk_sb)):
                    for ft in range(2):
                        ps = psum.tile([P, SC], F32, tag="mm_ps", name="mm_ps")
                        for ko in range(KO):
                            nc.tensor.matmul(
                                ps[:],
                                w_sb[:, ko, ft * P:(ft + 1) * P],
                                xt[:, ko, :],
                                start=(ko == 0),
                                stop=(ko == KO - 1),
                            )
                        # PSUM -> SBUF eviction with per-feature bias (DVE
                        # tensor_scalar: scalar operand is per-partition).
                        bcol = 2 * qk + ft
                        if qk == 1:
                            nc.vector.tensor_scalar_add(
                                kT_sb[ft][:, sc * SC:(sc + 1) * SC],
                                ps[:],
                                bqk_sb[:, bcol:bcol + 1],
                            )
                        else:
                            for hr in range(2):
                                rr = slice(hr * HD, (hr + 1) * HD)
                                nc.vector.tensor_scalar_add(
                                    qT_sb[2 * ft + hr][rr, sc * SC:(sc + 1) * SC],
                                    ps[rr, :],
                                    bqk_sb[rr, bcol:bcol + 1],
                                )
                for st in range(SC // P):
                    g_s = sc * (SC // P) + st
                    ps = psum.tile([P, SC], F32, tag="mm_ps", name="mm_ps")[:, :FG]
                    for ko in range(KO):
                        nc.tensor.matmul(
                            ps[:],
                            xt[:, ko, st * P:(st + 1) * P],
                            wv_sb[:, ko, :],
                            start=(ko == 0),
                            stop=(ko == KO - 1),
                        )
                    nc.vector.memset(v_sb[g_s][:, :, HD:VW], 1.0)
                    for h in range(HG):
                        nc.vector.tensor_tensor(
                            v_sb[g_s][:, h, 0:HD],
                            ps[:, h * HD:(h + 1) * HD],
                            bv_sb[:, h * HD:(h + 1) * HD],
                            mybir.AluOpType.add,
                        )

                # ---- causal attention; AllGather piece pc ships right
                # after its head pair so only the last piece is exposed ----
                for pc in range(2):
                    for hr in range(2):
                        h = 2 * pc + hr
                        hh = h // 2
                        n_j = (sc + 1) * (SC // P)
                        pv = psum.tile([P, SC], F32, tag="pv")
                        for jt in range(n_j):
                            o = jt - 4 * sc
                            off = max(0, 128 * o)  # diagonal blocks: skip i < j
                            sp = psum.tile([P, SC], F32, tag="score", name="sp")
                            nc.tensor.matmul(
                                sp[:, off:],
                                kT_sb[hh][:, jt * P:(jt + 1) * P],
                                qT_sb[h][:, sc * SC + off:(sc + 1) * SC],
                                start=True,
                                stop=True,
                            )
                            pt = pt_pool.tile([P, SC], MM_DT, tag="pt")
                            # exp(scores / sqrt(HD)); scores are bounded, no max
                            nc.scalar.activation(
                                pt[:, off:], sp[:, off:],
                                mybir.ActivationFunctionType.Exp,
                                scale=1.0 / float(np.sqrt(HD)),
                            )
                            if o >= 0:
                                # in-band causal mask: only the first 128
                                # columns of a diagonal block are staircase,
                                # the rest are all-allowed
                                nc.vector.tensor_tensor(
                                    pt[:, off:off + P], pt[:, off:off + P],
                                    maskw_sb[:],
                                    mybir.AluOpType.mult,
                                )
                            nc.tensor.matmul(
                                pv[:VW, off:],
                                v_sb[jt][:, h, :],
                                pt[:, off:],
                                start=(jt == 0),
                                stop=(jt == n_j - 1),
                            )
                        lrow = small.tile([1, SC], F32, tag="lrow")
                        nc.vector.tensor_copy(lrow[:], pv[HD:HD + 1, :])
                        rec = small.tile([1, SC], F32, tag="rec")
                        nc.vector.reciprocal_approx_fast(rec[:], lrow[:])
                        rec_b = small.tile([1, SC], MM_DT, tag="rec_b")
                        nc.vector.tensor_copy(rec_b[:], rec[:])
                        rb = psum.tile([P, SC], F32, tag="score", name="rb")
                        nc.tensor.matmul(rb[:HD, :], ones64[:], rec_b[:],
                                         start=True, stop=True)
                        rbs = small.tile([HD, SC], F32, tag="rbs")
                        nc.vector.tensor_copy(rbs[:], rb[:HD, :])
                        nc.vector.tensor_tensor(
                            aT_sb[h][0:HD, sc * SC:(sc + 1) * SC],
                            pv[0:HD, :],
                            rbs[:],
                            mybir.AluOpType.mult,
                        )

                    for hr in range(2):
                        h = 2 * pc + hr
                        nc.sync.dma_start(
                            aT_loc[sc][pc][hr * HD:(hr + 1) * HD, :],
                            aT_sb[h][0:HD, sc * SC:(sc + 1) * SC],
                        )
                    nc.gpsimd.collective_compute(
                        "AllGather",
                        mybir.AluOpType.bypass,
                        ins=[aT_loc[sc][pc][:].opt()],
                        outs=[aT_full[sc][pc][:].opt()],
                        replica_groups=REPLICA_GROUPS,
                    )

                if sc >= 1:
                    proj_chunk(sc - 1)
            proj_chunk(NSC - 1)
    return nc


_NC_CACHE = None


def _get_nc():
    global _NC_CACHE
    if _NC_CACHE is None:
        nc = bacc.Bacc("TRN2", target_bir_lowering=False, debug=False,
                       num_devices=N_CORES)
        build(nc)
        nc.compile()
        _NC_CACHE = nc
    return _NC_CACHE


def make_in_maps(x, c_attn_w, c_attn_b, c_proj_w, c_proj_b):
    x = np.asarray(x, dtype=np.float32)
    c_attn_w = np.asarray(c_attn_w, dtype=np.float32)
    c_attn_b = np.asarray(c_attn_b, dtype=np.float32)
    c_proj_w = np.asarray(c_proj_w, dtype=np.float32)
    c_proj_b = np.asarray(c_proj_b, dtype=np.float32)

    bf16 = ml_dtypes.bfloat16
    r = np.arange(P)[:, None]
    c = np.arange(P)[None, :]
    maskw = (c >= r).astype(np.float32)

    in_maps = []
    for core in range(N_CORES):
        b, g = divmod(core, NG)
        fsl = slice(g * FG, (g + 1) * FG)
        bq = c_attn_b[0 * NX:1 * NX][fsl]
        bk = c_attn_b[1 * NX:2 * NX][fsl]
        in_maps.append({
            "xT": np.ascontiguousarray(x[b].T).astype(bf16),
            "wq": np.ascontiguousarray(c_attn_w[:, 0 * NX:1 * NX][:, fsl]).astype(bf16),
            "wk": np.ascontiguousarray(c_attn_w[:, 1 * NX:2 * NX][:, fsl]).astype(bf16),
            "wv": np.ascontiguousarray(c_attn_w[:, 2 * NX:3 * NX][:, fsl]).astype(bf16),
            "bqk": np.stack([bq[0:P], bq[P:2 * P], bk[0:P], bk[P:2 * P]], axis=1)
                     .astype(np.float32).copy(),
            "bv": np.repeat(c_attn_b[2 * NX:3 * NX][fsl][None, :], P, axis=0).astype(bf16),
            "wp": np.ascontiguousarray(c_proj_w[:, fsl]).astype(bf16),
            "bp": np.ascontiguousarray(c_proj_b[fsl].reshape(2, P).T).astype(np.float32),
            "maskw": maskw.astype(bf16),
        })
    return in_maps


def assemble(results):
    """[core]{'out': [FG, S]} -> [B, S, NX] by transpose + concatenation."""
    full = np.empty((B, S, NX), dtype=np.float32)
    for core in range(N_CORES):
        b, g = divmod(core, NG)
        full[b, :, g * FG:(g + 1) * FG] = results[core]["out"].T
    return full


def kernel(x, c_attn_w, c_attn_b, c_proj_w, c_proj_b):
    nc = _get_nc()
    in_maps = make_in_maps(x, c_attn_w, c_attn_b, c_proj_w, c_proj_b)
    res = run_bass_kernel_spmd(nc, in_maps, core_ids=list(range(N_CORES)))
    return assemble(res.results)


# revision 8
# speedup vs baseline: 1.0332x; 1.0332x over previous
"""Distributed causal multi-head attention for 8 TRN2 NeuronCores.

Problem: y = (softmax(mask(Q K^T / sqrt(d))) V) @ c_proj_w + c_proj_b with
Q,K,V = split(x @ c_attn_w + c_attn_b), shapes B=2, S=2048, NX=1024, NH=16,
HD=64.

Sharding: core c = (b, g) with b = c // 4, g = c % 4 — data parallel over the
batch, tensor parallel over 4 head-groups of 4 heads. Each core:
  1. computes qT/kT ([d, s] layout) and v ([s, d] layout) for its 4 heads from
     a host-pretransposed x[b]^T, so no on-device transposes are ever needed;
  2. runs causal attention in the "S^T" orientation: scores come out of the PE
     as [j, i] tiles, exp() is fused into the PSUM->SBUF copy on the scalar
     engine (no max-subtraction — scores are bounded), and the softmax
     denominator falls out of the PV matmul for free via a ones-column
     appended to V;
  3. AllGathers aT per head-pair piece as soon as the pair finishes, and
     computes a 256-wide slice of the output projection in the transposed
     orientation (out^T = wp^T aT, [f, s] tiles), so each AG piece is exactly
     one 128-row contraction block and proj streams full 512-wide columns.
The host wrapper only slices/transposes inputs and concatenates outputs.
"""

import ml_dtypes
import numpy as np

import concourse.bass as bass
import concourse.mybir as mybir
from concourse import bacc, tile
from concourse.bass_utils import run_bass_kernel_spmd

B, S, NX, NH, HD = 2, 2048, 1024, 16, 64
NG = 4              # head-groups == cores per batch entry
HG = NH // NG       # heads per core
FG = HG * HD        # local feature width (256)
P = 128
SC = 512            # sequence chunk width
NSC = S // SC       # 4 chunks
KO = NX // P        # 8 contraction tiles
VW = HD + 1         # v tile width: 64 data cols + ones column
N_CORES = 8

F32 = mybir.dt.float32
MM_DT = mybir.dt.bfloat16

REPLICA_GROUPS = [[0, 1, 2, 3], [4, 5, 6, 7]]


def build(nc: bass.Bass):
    xT = nc.declare_dram_parameter("xT", [NX, S], MM_DT, isOutput=False)
    wq = nc.declare_dram_parameter("wq", [NX, FG], MM_DT, isOutput=False)
    wk = nc.declare_dram_parameter("wk", [NX, FG], MM_DT, isOutput=False)
    wv = nc.declare_dram_parameter("wv", [NX, FG], MM_DT, isOutput=False)
    bqk = nc.declare_dram_parameter("bqk", [P, 4], F32, isOutput=False)
    bv = nc.declare_dram_parameter("bv", [P, FG], MM_DT, isOutput=False)
    wp = nc.declare_dram_parameter("wp", [NX, FG], MM_DT, isOutput=False)
    bp = nc.declare_dram_parameter("bp", [P, 2], F32, isOutput=False)
    maskw = nc.declare_dram_parameter("maskw", [P, P], MM_DT, isOutput=False)
    out = nc.declare_dram_parameter("out", [FG, S], F32, isOutput=True)

    # Per-piece collective bounce buffers (collectives can't touch kernel I/O).
    # Piece (sc, pc) carries heads {2pc, 2pc+1} of chunk sc; after the
    # 4-rank AllGather, rank r's rows are global heads {4r+2pc, 4r+2pc+1}
    # == contraction tile ko = 2r+pc of the output projection.
    ag_warm_in = nc.dram_tensor("ag_warm_in", [4, 128], MM_DT)
    ag_warm_out = nc.dram_tensor("ag_warm_out", [16, 128], MM_DT)
    aT_loc = [[nc.dram_tensor(f"aT_loc{c}_{p_}", [P, SC], MM_DT)
               for p_ in range(2)] for c in range(NSC)]
    aT_full = [[nc.dram_tensor(f"aT_full{c}_{p_}", [NG * P, SC], MM_DT)
                for p_ in range(2)] for c in range(NSC)]

    with tile.TileContext(nc) as tc:
        nc_lp = nc.allow_low_precision(reason="bf16 PE compute path")
        nc_lp.__enter__()
        with (
            tc.tile_pool(name="consts", bufs=1) as consts,
            tc.tile_pool(name="persist", bufs=1) as persist,
            tc.tile_pool(name="xt", bufs=4) as xt_pool,
            tc.tile_pool(name="pt", bufs=8) as pt_pool,
            tc.tile_pool(name="aTf", bufs=2) as aTf_pool,
            tc.tile_pool(name="outs", bufs=3) as out_pool,
            tc.tile_pool(name="small", bufs=2) as small,
            tc.tile_pool(name="psum", bufs=2, space="PSUM") as psum,
        ):
            wq_sb = consts.tile([P, KO, FG], MM_DT)
            wk_sb = consts.tile([P, KO, FG], MM_DT)
            wv_sb = consts.tile([P, KO, FG], MM_DT)
            wp_sb = consts.tile([P, KO, FG], MM_DT)
            bqk_sb = consts.tile([P, 4], F32)
            bv_sb = consts.tile([P, FG], MM_DT)
            bp_sb = consts.tile([P, 2], F32)
            maskw_sb = consts.tile([P, P], MM_DT)
            ones64 = consts.tile([1, HD], MM_DT)

            # ---- startup: collective warm-up first on the CC path, then
            # the chunk-0-critical DMAs spread across engine queues so the
            # first QKV matmul isn't serialized behind a single ring ----
            nc.gpsimd.collective_compute(
                "AllGather",
                mybir.AluOpType.bypass,
                ins=[ag_warm_in[:].opt()],
                outs=[ag_warm_out[:].opt()],
                replica_groups=REPLICA_GROUPS,
            )
            xts = []
            for sc in range(NSC):
                xt = xt_pool.tile([P, KO, SC], MM_DT, tag="xt", name=f"xt{sc}")
                xts.append(xt)
            nc.sync.dma_start(
                xts[0][:], xT.rearrange("(ko p) s -> p ko s", p=P)[:, :, 0:SC]
            )
            nc.scalar.dma_start(wq_sb[:], wq.rearrange("(ko p) f -> p ko f", p=P))
            nc.scalar.dma_start(wk_sb[:], wk.rearrange("(ko p) f -> p ko f", p=P))
            nc.gpsimd.dma_start(wv_sb[:], wv.rearrange("(ko p) f -> p ko f", p=P))
            nc.gpsimd.dma_start(bqk_sb[:], bqk[:])
            nc.gpsimd.dma_start(bv_sb[:], bv[:])
            for sc in range(1, NSC):
                nc.sync.dma_start(
                    xts[sc][:],
                    xT.rearrange("(ko p) s -> p ko s", p=P)[:, :, sc * SC:(sc + 1) * SC],
                )
            nc.gpsimd.dma_start(maskw_sb[:], maskw[:])
            nc.gpsimd.dma_start(wp_sb[:], wp.rearrange("(ko p) f -> p ko f", p=P))
            nc.gpsimd.dma_start(bp_sb[:], bp[:])
            nc.vector.memset(ones64[:], 1.0)

            # ---- persistent activation tiles ----
            # kT: [d, s] packed — tile hh holds heads (2hh, 2hh+1) on
            # partition halves; it is the scores lhsT.
            # qT: one zero-padded [128, s] tile per head, data on the same
            # partition half as in kT, zeros elsewhere — the zeros select
            # the head out of the packed kT during the scores matmul.
            # v: [s, 65] per head per 128-row tile: 64 data cols + a ones
            # column (softmax denominator); the PV lhsT is [128, 65] so no
            # zero padding is ever needed.
            # aT: per-head [128, s]; only rows 0:64 are meaningful.
            qT_sb = [persist.tile([P, S], MM_DT, name=f"qT{h}") for h in range(HG)]
            kT_sb = [persist.tile([P, S], MM_DT, name=f"kT{hh}") for hh in range(2)]
            v_sb = [persist.tile([P, HG, VW], MM_DT, name=f"v{st}")
                    for st in range(S // P)]
            aT_sb = [persist.tile([P, S], MM_DT, name=f"aT{h}") for h in range(HG)]
            for h in range(HG):
                pad0 = (1 - h % 2) * HD
                nc.vector.memset(qT_sb[h][pad0:pad0 + HD, :], 0.0)

            def proj_chunk(sc):
                # piece loads go on the gpsimd queue: a load blocked on its
                # AllGather must not head-of-line-block the sync queue where
                # the next chunk's aT piece stores live
                aTfp = []
                for pc in range(2):
                    t = aTf_pool.tile([P, NG, SC], MM_DT, tag=f"aTfp{pc}",
                                      name=f"aTfp{pc}")
                    nc.gpsimd.dma_start(
                        t[:], aT_full[sc][pc].rearrange("(r p) s -> p r s", p=P)
                    )
                    aTfp.append(t)
                ps_ft = [psum.tile([P, SC], F32, tag="proj", name=f"proj{ft}")
                         for ft in range(2)]
                for pc in range(2):
                    for ft in range(2):
                        for r in range(NG):
                            nc.tensor.matmul(
                                ps_ft[ft][:],
                                wp_sb[:, 2 * r + pc, ft * P:(ft + 1) * P],
                                aTfp[pc][:, r, :],
                                start=(pc == 0 and r == 0),
                                stop=(pc == 1 and r == NG - 1),
                            )
                for ft in range(2):
                    ot = out_pool.tile([P, SC], F32, tag="ot")
                    nc.vector.tensor_scalar_add(
                        ot[:], ps_ft[ft][:], bp_sb[:, ft:ft + 1]
                    )
                    nc.sync.dma_start(
                        out[ft * P:(ft + 1) * P, sc * SC:(sc + 1) * SC], ot[:]
                    )

            # ===== per-chunk pipeline: QKV -> attention -> AllGather pieces
            # -> (deferred one chunk) output projection =====
            for sc in range(NSC):
                # ---- QKV for this chunk ----
                xt = xts[sc]
                for qk, w_sb in enumerate((wq_sb, wk_sb)):
                    for ft in range(2):
                        ps = psum.tile([P, SC], F32, tag="mm_ps", name="mm_ps")
                        for ko in range(KO):
                            nc.tensor.matmul(
                                ps[:],
                                w_sb[:, ko, ft * P:(ft + 1) * P],
                                xt[:, ko, :],
                                start=(ko == 0),
                                stop=(ko == KO - 1),
                            )
                        # PSUM -> SBUF eviction with per-feature bias (DVE
                        # tensor_scalar: scalar operand is per-partition).
                        bcol = 2 * qk + ft
                        if qk == 1:
                            nc.vector.tensor_scalar_add(
                                kT_sb[ft][:, sc * SC:(sc + 1) * SC],
                                ps[:],
                                bqk_sb[:, bcol:bcol + 1],
                            )
                        else:
                            for hr in range(2):
                                rr = slice(hr * HD, (hr + 1) * HD)
                                nc.vector.tensor_scalar_add(
                                    qT_sb[2 * ft + hr][rr, sc * SC:(sc + 1) * SC],
                                    ps[rr, :],
                                    bqk_sb[rr, bcol:bcol + 1],
                                )
                for st in range(SC // P):
                    g_s = sc * (SC // P) + st
                    ps = psum.tile([P, SC], F32, tag="mm_ps", name="mm_ps")[:, :FG]
                    for ko in range(KO):
                        nc.tensor.matmul(
                            ps[:],
                            xt[:, ko, st * P:(st + 1) * P],
                            wv_sb[:, ko, :],
                            start=(ko == 0),
                            stop=(ko == KO - 1),
                        )
                    nc.vector.memset(v_sb[g_s][:, :, HD:VW], 1.0)
                    for h in range(HG):
                        nc.vector.tensor_tensor(
                            v_sb[g_s][:, h, 0:HD],
                            ps[:, h * HD:(h + 1) * HD],
                            bv_sb[:, h * HD:(h + 1) * HD],
                            mybir.AluOpType.add,
                        )

                # ---- causal attention. Normalization of head h is deferred
                # until after head h+1's score/PV chain is emitted, so the
                # serial reciprocal->broadcast vector chain runs while the PE
                # streams the next head's matmuls. AllGather piece pc ships
                # as soon as its two heads are normalized. ----
                def pv_chain(h):
                    hh = h // 2
                    n_j = (sc + 1) * (SC // P)
                    pv = psum.tile([P, SC], F32, tag="pv", name="pv")
                    for jt in range(n_j):
                        o = jt - 4 * sc
                        off = max(0, 128 * o)  # diagonal blocks: skip i < j
                        sp = psum.tile([P, SC], F32, tag="score", name="sp")
                        nc.tensor.matmul(
                            sp[:, off:],
                            kT_sb[hh][:, jt * P:(jt + 1) * P],
                            qT_sb[h][:, sc * SC + off:(sc + 1) * SC],
                            start=True,
                            stop=True,
                        )
                        pt = pt_pool.tile([P, SC], MM_DT, tag="pt")
                        # exp(scores / sqrt(HD)); scores are bounded, no max
                        nc.scalar.activation(
                            pt[:, off:], sp[:, off:],
                            mybir.ActivationFunctionType.Exp,
                            scale=1.0 / float(np.sqrt(HD)),
                        )
                        if o >= 0:
                            # in-band causal mask: only the first 128
                            # columns of a diagonal block are staircase,
                            # the rest are all-allowed
                            nc.vector.tensor_tensor(
                                pt[:, off:off + P], pt[:, off:off + P],
                                maskw_sb[:],
                                mybir.AluOpType.mult,
                            )
                        nc.tensor.matmul(
                            pv[:VW, off:],
                            v_sb[jt][:, h, :],
                            pt[:, off:],
                            start=(jt == 0),
                            stop=(jt == n_j - 1),
                        )
                    return pv

                def norm_head(h, pv):
                    lrow = small.tile([1, SC], F32, tag="lrow")
                    nc.vector.tensor_copy(lrow[:], pv[HD:HD + 1, :])
                    rec = small.tile([1, SC], F32, tag="rec")
                    nc.vector.reciprocal_approx_fast(rec[:], lrow[:])
                    rec_b = small.tile([1, SC], MM_DT, tag="rec_b")
                    nc.vector.tensor_copy(rec_b[:], rec[:])
                    rb = psum.tile([P, SC], F32, tag="score", name="rb")
                    nc.tensor.matmul(rb[:HD, :], ones64[:], rec_b[:],
                                     start=True, stop=True)
                    rbs = small.tile([HD, SC], F32, tag="rbs")
                    nc.vector.tensor_copy(rbs[:], rb[:HD, :])
                    nc.vector.tensor_tensor(
                        aT_sb[h][0:HD, sc * SC:(sc + 1) * SC],
                        pv[0:HD, :],
                        rbs[:],
                        mybir.AluOpType.mult,
                    )

                def ship_piece(pc):
                    for hr in range(2):
                        h = 2 * pc + hr
                        nc.sync.dma_start(
                            aT_loc[sc][pc][hr * HD:(hr + 1) * HD, :],
                            aT_sb[h][0:HD, sc * SC:(sc + 1) * SC],
                        )
                    nc.gpsimd.collective_compute(
                        "AllGather",
                        mybir.AluOpType.bypass,
                        ins=[aT_loc[sc][pc][:].opt()],
                        outs=[aT_full[sc][pc][:].opt()],
                        replica_groups=REPLICA_GROUPS,
                    )

                pvs = []
                for h in range(HG):
                    pvs.append(pv_chain(h))
                    if h >= 1:
                        norm_head(h - 1, pvs[h - 1])
                    if h == 2:
                        ship_piece(0)
                if sc >= 1:
                    proj_chunk(sc - 1)
                norm_head(3, pvs[3])
                ship_piece(1)
            proj_chunk(NSC - 1)
    return nc


_NC_CACHE = None


def _get_nc():
    global _NC_CACHE
    if _NC_CACHE is None:
        nc = bacc.Bacc("TRN2", target_bir_lowering=False, debug=False,
                       num_devices=N_CORES)
        build(nc)
        nc.compile()
        _NC_CACHE = nc
    return _NC_CACHE


def make_in_maps(x, c_attn_w, c_attn_b, c_proj_w, c_proj_b):
    x = np.asarray(x, dtype=np.float32)
    c_attn_w = np.asarray(c_attn_w, dtype=np.float32)
    c_attn_b = np.asarray(c_attn_b, dtype=np.float32)
    c_proj_w = np.asarray(c_proj_w, dtype=np.float32)
    c_proj_b = np.asarray(c_proj_b, dtype=np.float32)

    bf16 = ml_dtypes.bfloat16
    r = np.arange(P)[:, None]
    c = np.arange(P)[None, :]
    maskw = (c >= r).astype(np.float32)

    in_maps = []
    for core in range(N_CORES):
        b, g = divmod(core, NG)
        fsl = slice(g * FG, (g + 1) * FG)
        bq = c_attn_b[0 * NX:1 * NX][fsl]
        bk = c_attn_b[1 * NX:2 * NX][fsl]
        in_maps.append({
            "xT": np.ascontiguousarray(x[b].T).astype(bf16),
            "wq": np.ascontiguousarray(c_attn_w[:, 0 * NX:1 * NX][:, fsl]).astype(bf16),
            "wk": np.ascontiguousarray(c_attn_w[:, 1 * NX:2 * NX][:, fsl]).astype(bf16),
            "wv": np.ascontiguousarray(c_attn_w[:, 2 * NX:3 * NX][:, fsl]).astype(bf16),
            "bqk": np.stack([bq[0:P], bq[P:2 * P], bk[0:P], bk[P:2 * P]], axis=1)
                     .astype(np.float32).copy(),
            "bv": np.repeat(c_attn_b[2 * NX:3 * NX][fsl][None, :], P, axis=0).astype(bf16),
            "wp": np.ascontiguousarray(c_proj_w[:, fsl]).astype(bf16),
            "bp": np.ascontiguousarray(c_proj_b[fsl].reshape(2, P).T).astype(np.float32),
            "maskw": maskw.astype(bf16),
        })
    return in_maps


def assemble(results):
    """[core]{'out': [FG, S]} -> [B, S, NX] by transpose + concatenation."""
    full = np.empty((B, S, NX), dtype=np.float32)
    for core in range(N_CORES):
        b, g = divmod(core, NG)
        full[b, :, g * FG:(g + 1) * FG] = results[core]["out"].T
    return full


def kernel(x, c_attn_w, c_attn_b, c_proj_w, c_proj_b):
    nc = _get_nc()
    in_maps = make_in_maps(x, c_attn_w, c_attn_b, c_proj_w, c_proj_b)
    res = run_bass_kernel_spmd(nc, in_maps, core_ids=list(range(N_CORES)))
    return assemble(res.results)


# revision 11
# speedup vs baseline: 1.1414x; 1.1047x over previous
"""Distributed causal multi-head attention for 8 TRN2 NeuronCores.

Problem: y = (softmax(mask(Q K^T / sqrt(d))) V) @ c_proj_w + c_proj_b with
Q,K,V = split(x @ c_attn_w + c_attn_b), shapes B=2, S=2048, NX=1024, NH=16,
HD=64.

Sharding: core c = (b, g) with b = c // 4, g = c % 4 — data parallel over the
batch, tensor parallel over 4 head-groups of 4 heads. Each core:
  1. computes qT/kT ([d, s] layout) and v ([s, d] layout) for its 4 heads from
     a host-pretransposed x[b]^T, so no on-device transposes are ever needed;
  2. runs causal attention in the "S^T" orientation: scores come out of the PE
     as [j, i] tiles, exp() is fused into the PSUM->SBUF copy on the scalar
     engine (no max-subtraction — scores are bounded), and the softmax
     denominator falls out of the PV matmul for free via a ones-column
     appended to V;
  3. AllGathers aT per head-pair piece as soon as the pair finishes, and
     computes a 256-wide slice of the output projection in the transposed
     orientation (out^T = wp^T aT, [f, s] tiles), so each AG piece is exactly
     one 128-row contraction block and proj streams full 512-wide columns.
The host wrapper only slices/transposes inputs and concatenates outputs.
"""

import ml_dtypes
import numpy as np

import concourse.bass as bass
import concourse.mybir as mybir
from concourse import bacc, tile
from concourse.bass_utils import run_bass_kernel_spmd

B, S, NX, NH, HD = 2, 2048, 1024, 16, 64
NG = 4              # head-groups == cores per batch entry
HG = NH // NG       # heads per core
FG = HG * HD        # local feature width (256)
P = 128
SC = 512            # sequence chunk width
NSC = S // SC       # 4 chunks
KO = NX // P        # 8 contraction tiles
VW = HD + 1         # v tile width: 64 data cols + ones column
N_CORES = 8

F32 = mybir.dt.float32
MM_DT = mybir.dt.bfloat16

REPLICA_GROUPS = [[0, 1, 2, 3], [4, 5, 6, 7]]


def build(nc: bass.Bass):
    xT = nc.declare_dram_parameter("xT", [NX, S], MM_DT, isOutput=False)
    wq = nc.declare_dram_parameter("wq", [NX, FG], MM_DT, isOutput=False)
    wk = nc.declare_dram_parameter("wk", [NX, FG], MM_DT, isOutput=False)
    wv = nc.declare_dram_parameter("wv", [NX, FG], MM_DT, isOutput=False)
    bqk = nc.declare_dram_parameter("bqk", [P, 4], F32, isOutput=False)
    bv = nc.declare_dram_parameter("bv", [P, FG], MM_DT, isOutput=False)
    wp = nc.declare_dram_parameter("wp", [NX, FG], MM_DT, isOutput=False)
    bp = nc.declare_dram_parameter("bp", [P, 2], F32, isOutput=False)
    maskw = nc.declare_dram_parameter("maskw", [P, P], MM_DT, isOutput=False)
    out = nc.declare_dram_parameter("out", [FG, S], F32, isOutput=True)

    # Per-piece collective bounce buffers (collectives can't touch kernel I/O).
    # Piece (sc, pc) carries heads {2pc, 2pc+1} of chunk sc; after the
    # 4-rank AllGather, rank r's rows are global heads {4r+2pc, 4r+2pc+1}
    # == contraction tile ko = 2r+pc of the output projection.
    ag_warm_in = nc.dram_tensor("ag_warm_in", [4, 128], MM_DT)
    ag_warm_out = nc.dram_tensor("ag_warm_out", [16, 128], MM_DT)
    # chunks 0..NSC-2: one AllGather per chunk (all 4 local heads);
    # last chunk: two head-pair pieces so only the final piece's latency
    # is exposed at the tail
    aT_loc1 = [nc.dram_tensor(f"aT_locS{c}", [FG, SC], MM_DT)
               for c in range(NSC - 1)]
    aT_full1 = [nc.dram_tensor(f"aT_fullS{c}", [NG * FG, SC], MM_DT)
                for c in range(NSC - 1)]
    aT_loc = [nc.dram_tensor(f"aT_loc{p_}", [P, SC], MM_DT) for p_ in range(2)]
    aT_full = [nc.dram_tensor(f"aT_full{p_}", [NG * P, SC], MM_DT)
               for p_ in range(2)]

    with tile.TileContext(nc) as tc:
        nc_lp = nc.allow_low_precision(reason="bf16 PE compute path")
        nc_lp.__enter__()
        with (
            tc.tile_pool(name="consts", bufs=1) as consts,
            tc.tile_pool(name="persist", bufs=1) as persist,
            tc.tile_pool(name="xt", bufs=4) as xt_pool,
            tc.tile_pool(name="pt", bufs=8) as pt_pool,
            tc.tile_pool(name="aTf", bufs=2) as aTf_pool,
            tc.tile_pool(name="outs", bufs=3) as out_pool,
            tc.tile_pool(name="small", bufs=2) as small,
            tc.tile_pool(name="psum", bufs=2, space="PSUM") as psum,
        ):
            wq_sb = consts.tile([P, KO, FG], MM_DT)
            wk_sb = consts.tile([P, KO, FG], MM_DT)
            wv_sb = consts.tile([P, KO, FG], MM_DT)
            wp_sb = consts.tile([P, KO, FG], MM_DT)
            bqk_sb = consts.tile([P, 4], F32)
            bv_sb = consts.tile([P, FG], MM_DT)
            bp_sb = consts.tile([P, 2], F32)
            maskw_sb = consts.tile([P, P], MM_DT)
            ones64 = consts.tile([1, HD], MM_DT)

            # ---- startup: collective warm-up first on the CC path, then
            # the chunk-0-critical DMAs spread across engine queues so the
            # first QKV matmul isn't serialized behind a single ring ----
            nc.gpsimd.collective_compute(
                "AllGather",
                mybir.AluOpType.bypass,
                ins=[ag_warm_in[:].opt()],
                outs=[ag_warm_out[:].opt()],
                replica_groups=REPLICA_GROUPS,
            )
            xts = []
            for sc in range(NSC):
                xt = xt_pool.tile([P, KO, SC], MM_DT, tag="xt", name=f"xt{sc}")
                xts.append(xt)
            nc.sync.dma_start(
                xts[0][:], xT.rearrange("(ko p) s -> p ko s", p=P)[:, :, 0:SC]
            )
            nc.scalar.dma_start(wq_sb[:], wq.rearrange("(ko p) f -> p ko f", p=P))
            nc.scalar.dma_start(wk_sb[:], wk.rearrange("(ko p) f -> p ko f", p=P))
            nc.gpsimd.dma_start(wv_sb[:], wv.rearrange("(ko p) f -> p ko f", p=P))
            nc.gpsimd.dma_start(bqk_sb[:], bqk[:])
            nc.gpsimd.dma_start(bv_sb[:], bv[:])
            for sc in range(1, NSC):
                nc.sync.dma_start(
                    xts[sc][:],
                    xT.rearrange("(ko p) s -> p ko s", p=P)[:, :, sc * SC:(sc + 1) * SC],
                )
            nc.gpsimd.dma_start(maskw_sb[:], maskw[:])
            nc.gpsimd.dma_start(wp_sb[:], wp.rearrange("(ko p) f -> p ko f", p=P))
            nc.gpsimd.dma_start(bp_sb[:], bp[:])
            nc.vector.memset(ones64[:], 1.0)

            # ---- persistent activation tiles ----
            # kT: [d, s] packed — tile hh holds heads (2hh, 2hh+1) on
            # partition halves; it is the scores lhsT.
            # qT: one zero-padded [128, s] tile per head, data on the same
            # partition half as in kT, zeros elsewhere — the zeros select
            # the head out of the packed kT during the scores matmul.
            # v: [s, 65] per head per 128-row tile: 64 data cols + a ones
            # column (softmax denominator); the PV lhsT is [128, 65] so no
            # zero padding is ever needed.
            # aT: per-head [128, s]; only rows 0:64 are meaningful.
            qT_sb = [persist.tile([P, S], MM_DT, name=f"qT{h}") for h in range(HG)]
            kT_sb = [persist.tile([P, S], MM_DT, name=f"kT{hh}") for hh in range(2)]
            v_sb = [persist.tile([P, HG, VW], MM_DT, name=f"v{st}")
                    for st in range(S // P)]
            aT_sb = [persist.tile([P, S], MM_DT, name=f"aT{h}") for h in range(HG)]
            for h in range(HG):
                pad0 = (1 - h % 2) * HD
                nc.vector.memset(qT_sb[h][pad0:pad0 + HD, :], 0.0)

            def proj_chunk(sc):
                # piece loads go on the gpsimd queue: a load blocked on its
                # AllGather must not head-of-line-block the sync queue where
                # the next chunk's aT piece stores live
                ps_ft = [psum.tile([P, SC], F32, tag="proj", name=f"proj{ft}")
                         for ft in range(2)]
                if sc < NSC - 1:
                    aTf = aTf_pool.tile([P, KO, SC], MM_DT, tag="aTf", name="aTf")
                    nc.gpsimd.dma_start(
                        aTf[:], aT_full1[sc].rearrange("(ko p) s -> p ko s", p=P)
                    )
                    for ft in range(2):
                        for ko in range(KO):
                            nc.tensor.matmul(
                                ps_ft[ft][:],
                                wp_sb[:, ko, ft * P:(ft + 1) * P],
                                aTf[:, ko, :],
                                start=(ko == 0),
                                stop=(ko == KO - 1),
                            )
                else:
                    # last chunk: 2 head-pair pieces; rank r of piece pc is
                    # contraction tile ko = 2r+pc, so even kos complete with
                    # piece 0 and odd kos with piece 1
                    aTfp = []
                    for pc in range(2):
                        t = aTf_pool.tile([P, NG, SC], MM_DT, tag=f"aTfp{pc}",
                                          name=f"aTfp{pc}")
                        nc.gpsimd.dma_start(
                            t[:], aT_full[pc].rearrange("(r p) s -> p r s", p=P)
                        )
                        aTfp.append(t)
                    for pc in range(2):
                        for ft in range(2):
                            for r in range(NG):
                                nc.tensor.matmul(
                                    ps_ft[ft][:],
                                    wp_sb[:, 2 * r + pc, ft * P:(ft + 1) * P],
                                    aTfp[pc][:, r, :],
                                    start=(pc == 0 and r == 0),
                                    stop=(pc == 1 and r == NG - 1),
                                )
                for ft in range(2):
                    ot = out_pool.tile([P, SC], F32, tag="ot")
                    nc.vector.tensor_scalar_add(
                        ot[:], ps_ft[ft][:], bp_sb[:, ft:ft + 1]
                    )
                    nc.sync.dma_start(
                        out[ft * P:(ft + 1) * P, sc * SC:(sc + 1) * SC], ot[:]
                    )

            # ===== per-chunk pipeline: QKV -> attention -> AllGather pieces
            # -> (deferred one chunk) output projection =====
            for sc in range(NSC):
                # ---- QKV for this chunk ----
                xt = xts[sc]
                for qk, w_sb in enumerate((wq_sb, wk_sb)):
                    for ft in range(2):
                        ps = psum.tile([P, SC], F32, tag="mm_ps", name="mm_ps")
                        for ko in range(KO):
                            nc.tensor.matmul(
                                ps[:],
                                w_sb[:, ko, ft * P:(ft + 1) * P],
                                xt[:, ko, :],
                                start=(ko == 0),
                                stop=(ko == KO - 1),
                            )
                        # PSUM -> SBUF eviction with per-feature bias (DVE
                        # tensor_scalar: scalar operand is per-partition).
                        bcol = 2 * qk + ft
                        if qk == 1:
                            nc.vector.tensor_scalar_add(
                                kT_sb[ft][:, sc * SC:(sc + 1) * SC],
                                ps[:],
                                bqk_sb[:, bcol:bcol + 1],
                            )
                        else:
                            for hr in range(2):
                                rr = slice(hr * HD, (hr + 1) * HD)
                                nc.vector.tensor_scalar_add(
                                    qT_sb[2 * ft + hr][rr, sc * SC:(sc + 1) * SC],
                                    ps[rr, :],
                                    bqk_sb[rr, bcol:bcol + 1],
                                )
                for st in range(SC // P):
                    g_s = sc * (SC // P) + st
                    ps = psum.tile([P, SC], F32, tag="mm_ps", name="mm_ps")[:, :FG]
                    for ko in range(KO):
                        nc.tensor.matmul(
                            ps[:],
                            xt[:, ko, st * P:(st + 1) * P],
                            wv_sb[:, ko, :],
                            start=(ko == 0),
                            stop=(ko == KO - 1),
                        )
                    nc.vector.memset(v_sb[g_s][:, :, HD:VW], 1.0)
                    for h in range(HG):
                        nc.vector.tensor_tensor(
                            v_sb[g_s][:, h, 0:HD],
                            ps[:, h * HD:(h + 1) * HD],
                            bv_sb[:, h * HD:(h + 1) * HD],
                            mybir.AluOpType.add,
                        )

                # ---- causal attention. Normalization of head h is deferred
                # until after head h+1's score/PV chain is emitted, so the
                # serial reciprocal->broadcast vector chain runs while the PE
                # streams the next head's matmuls. AllGather piece pc ships
                # as soon as its two heads are normalized. ----
                def pv_chain(h):
                    hh = h // 2
                    n_j = (sc + 1) * (SC // P)
                    pv = psum.tile([P, SC], F32, tag="pv", name="pv")
                    for jt in range(n_j):
                        o = jt - 4 * sc
                        off = max(0, 128 * o)  # diagonal blocks: skip i < j
                        sp = psum.tile([P, SC], F32, tag="score", name="sp")
                        nc.tensor.matmul(
                            sp[:, off:],
                            kT_sb[hh][:, jt * P:(jt + 1) * P],
                            qT_sb[h][:, sc * SC + off:(sc + 1) * SC],
                            start=True,
                            stop=True,
                        )
                        pt = pt_pool.tile([P, SC], MM_DT, tag="pt")
                        # exp(scores / sqrt(HD)); scores are bounded, no max
                        nc.scalar.activation(
                            pt[:, off:], sp[:, off:],
                            mybir.ActivationFunctionType.Exp,
                            scale=1.0 / float(np.sqrt(HD)),
                        )
                        if o >= 0:
                            # in-band causal mask: only the first 128
                            # columns of a diagonal block are staircase,
                            # the rest are all-allowed
                            nc.vector.tensor_tensor(
                                pt[:, off:off + P], pt[:, off:off + P],
                                maskw_sb[:],
                                mybir.AluOpType.mult,
                            )
                        nc.tensor.matmul(
                            pv[:VW, off:],
                            v_sb[jt][:, h, :],
                            pt[:, off:],
                            start=(jt == 0),
                            stop=(jt == n_j - 1),
                        )
                    return pv

                def norm_head(h, pv):
                    lrow = small.tile([1, SC], F32, tag="lrow")
                    nc.vector.tensor_copy(lrow[:], pv[HD:HD + 1, :])
                    rec = small.tile([1, SC], F32, tag="rec")
                    nc.vector.reciprocal_approx_fast(rec[:], lrow[:])
                    rec_b = small.tile([1, SC], MM_DT, tag="rec_b")
                    nc.vector.tensor_copy(rec_b[:], rec[:])
                    rb = psum.tile([P, SC], F32, tag="score", name="rb")
                    nc.tensor.matmul(rb[:HD, :], ones64[:], rec_b[:],
                                     start=True, stop=True)
                    rbs = small.tile([HD, SC], F32, tag="rbs")
                    nc.vector.tensor_copy(rbs[:], rb[:HD, :])
                    nc.vector.tensor_tensor(
                        aT_sb[h][0:HD, sc * SC:(sc + 1) * SC],
                        pv[0:HD, :],
                        rbs[:],
                        mybir.AluOpType.mult,
                    )

                def ship_piece(pc):
                    for hr in range(2):
                        h = 2 * pc + hr
                        nc.sync.dma_start(
                            aT_loc[pc][hr * HD:(hr + 1) * HD, :],
                            aT_sb[h][0:HD, sc * SC:(sc + 1) * SC],
                        )
                    nc.gpsimd.collective_compute(
                        "AllGather",
                        mybir.AluOpType.bypass,
                        ins=[aT_loc[pc][:].opt()],
                        outs=[aT_full[pc][:].opt()],
                        replica_groups=REPLICA_GROUPS,
                    )

                def ship_chunk():
                    for h in range(HG):
                        nc.sync.dma_start(
                            aT_loc1[sc][h * HD:(h + 1) * HD, :],
                            aT_sb[h][0:HD, sc * SC:(sc + 1) * SC],
                        )
                    nc.gpsimd.collective_compute(
                        "AllGather",
                        mybir.AluOpType.bypass,
                        ins=[aT_loc1[sc][:].opt()],
                        outs=[aT_full1[sc][:].opt()],
                        replica_groups=REPLICA_GROUPS,
                    )

                last = sc == NSC - 1
                pvs = []
                for h in range(HG):
                    pvs.append(pv_chain(h))
                    if h >= 1:
                        norm_head(h - 1, pvs[h - 1])
                    if h == 2 and last:
                        ship_piece(0)
                if sc >= 1:
                    proj_chunk(sc - 1)
                norm_head(3, pvs[3])
                if last:
                    ship_piece(1)
                else:
                    ship_chunk()
            proj_chunk(NSC - 1)
    return nc


_NC_CACHE = None


def _get_nc():
    global _NC_CACHE
    if _NC_CACHE is None:
        nc = bacc.Bacc("TRN2", target_bir_lowering=False, debug=False,
                       num_devices=N_CORES)
        build(nc)
        nc.compile()
        _NC_CACHE = nc
    return _NC_CACHE


def make_in_maps(x, c_attn_w, c_attn_b, c_proj_w, c_proj_b):
    x = np.asarray(x, dtype=np.float32)
    c_attn_w = np.asarray(c_attn_w, dtype=np.float32)
    c_attn_b = np.asarray(c_attn_b, dtype=np.float32)
    c_proj_w = np.asarray(c_proj_w, dtype=np.float32)
    c_proj_b = np.asarray(c_proj_b, dtype=np.float32)

    bf16 = ml_dtypes.bfloat16
    r = np.arange(P)[:, None]
    c = np.arange(P)[None, :]
    maskw = (c >= r).astype(np.float32)

    in_maps = []
    for core in range(N_CORES):
        b, g = divmod(core, NG)
        fsl = slice(g * FG, (g + 1) * FG)
        bq = c_attn_b[0 * NX:1 * NX][fsl]
        bk = c_attn_b[1 * NX:2 * NX][fsl]
        in_maps.append({
            "xT": np.ascontiguousarray(x[b].T).astype(bf16),
            "wq": np.ascontiguousarray(c_attn_w[:, 0 * NX:1 * NX][:, fsl]).astype(bf16),
            "wk": np.ascontiguousarray(c_attn_w[:, 1 * NX:2 * NX][:, fsl]).astype(bf16),
            "wv": np.ascontiguousarray(c_attn_w[:, 2 * NX:3 * NX][:, fsl]).astype(bf16),
            "bqk": np.stack([bq[0:P], bq[P:2 * P], bk[0:P], bk[P:2 * P]], axis=1)
                     .astype(np.float32).copy(),
            "bv": np.repeat(c_attn_b[2 * NX:3 * NX][fsl][None, :], P, axis=0).astype(bf16),
            "wp": np.ascontiguousarray(c_proj_w[:, fsl]).astype(bf16),
            "bp": np.ascontiguousarray(c_proj_b[fsl].reshape(2, P).T).astype(np.float32),
            "maskw": maskw.astype(bf16),
        })
    return in_maps


def assemble(results):
    """[core]{'out': [FG, S]} -> [B, S, NX] by transpose + concatenation."""
    full = np.empty((B, S, NX), dtype=np.float32)
    for core in range(N_CORES):
        b, g = divmod(core, NG)
        full[b, :, g * FG:(g + 1) * FG] = results[core]["out"].T
    return full


def kernel(x, c_attn_w, c_attn_b, c_proj_w, c_proj_b):
    nc = _get_nc()
    in_maps = make_in_maps(x, c_attn_w, c_attn_b, c_proj_w, c_proj_b)
    res = run_bass_kernel_spmd(nc, in_maps, core_ids=list(range(N_CORES)))
    return assemble(res.results)
